# revision 15
# baseline (speedup 1.0000x reference)
"""Trainium2 Bass kernel for structured-sparse matmul.

Computes: out[b,s,o] = sum_k x[b,s,sparse_idx[k]] * sparse_values[o,k]
  x: [4, 2048, 4096] f32, sparse_values: [4096, 1024] f32,
  sparse_idx: [1024] int64 (sorted, unique) -> out [4, 2048, 4096] f32

Strategy (8 NeuronCores, data-parallel over rows m = b*s, bf16 compute):
  Host-side layout prep only (like the wt swizzle): x slice is transposed
  and swizzled to [chunk, part, n-block, m] bf16 so every DMA moves large
  contiguous per-partition lines.  sparse_idx expands into one-hot
  selection blocks G (compile-time metadata).
  Per core (M=1024 rows), per m-chunk of 256:
    gather n->k via PE matmuls with G (xg_T[k, m] bf16), then GEMM
    out[m, o] = xg_T.T @ W^T[k, o] accumulating 8 k-tiles in PSUM
    (bf16 = 1 col/cycle = the PE roofline here: fp8 DoubleRow measures
    2x but the 2e-2 accuracy gate forces a 3-term split = 1.5x bf16
    cost; GPSIMD ap_gather measures ~3 Gelem/s - 50x too slow), evict
    f32->bf16 (DVE, + ACT on the last chunk) and store bf16 per o-half
    (host upcasts).
  The G blocks are synthesized ON DEVICE (gpsimd iota + DVE is_equal
  against the replicated sparse_idx) so only 512KB of idx data rides the
  critical DMA prefix instead of 1.2MB of one-hot blocks, and wt0 can
  lead the scalar ring.  The PE clock (HAM-gated, 0.65->2.4 GHz over
  ~3.4us of activity) is kept warm by a long run of small matmuls on a
  gpsimd-memset tile - no DMA dependency - filling the whole x-load
  window so the real stream never re-ramps.
"""

import sys

if "/opt/trn_rl_repo" not in sys.path:
    sys.path.insert(0, "/opt/trn_rl_repo")

import numpy as np

B, S, N_IN = 4, 2048, 4096
N_OUT, N_SPARSE = 4096, 1024
N_CORES = 8
M_TOT = B * S            # 8192
M = M_TOT // N_CORES     # 1024 rows per core
P = 128
NKT = N_SPARSE // P      # 8 k-tiles
NNB = N_IN // P          # 32 n-blocks
MC = 256                 # m-chunk for gather/GEMM pipelining
NCH = M // MC            # 4 chunks
TPC = MC // P            # 2 m-tiles per chunk
O_TILE = 512
NOS = N_OUT // O_TILE    # 8 o-slices
N_WARM = 12

_cache: dict = {}


def _build_gather_blocks(idx: np.ndarray):
    """Expand sparse_idx into one-hot selection blocks.

    For k-tile kt and n-block b, G[n, krel] = 1 iff idx[kt*128+krel] == b*128+n.
    Returns (g_all [NB,128,128] f32, blocks_per_kt: list of lists of (bi, b)).
    """
    mats = []
    blocks_per_kt = []
    for kt in range(NKT):
        ks = idx[kt * P:(kt + 1) * P]
        bs = sorted(set(int(k) // P for k in ks))
        entries = []
        for b in bs:
            mat = np.zeros((P, P), dtype=np.float32)
            for krel, k in enumerate(ks):
                if int(k) // P == b:
                    mat[int(k) % P, krel] = 1.0
            entries.append((len(mats), b))
            mats.append(mat)
        blocks_per_kt.append(entries)
    return np.stack(mats), blocks_per_kt


def _build_nc(blocks_per_kt, nb_total):
    import concourse.mybir as mybir
    import concourse.tile as tile
    from concourse import bacc

    BF16 = mybir.dt.bfloat16
    F32 = mybir.dt.float32

    nc = bacc.Bacc("TRN2", target_bir_lowering=False, debug=False)
    # x swizzled on host: [chunk, part(n%128), n-block, m] bf16
    x = nc.dram_tensor("x", [NCH, P, NNB, MC], BF16, kind="ExternalInput")
    wt = nc.dram_tensor("wt", [NOS, P, NKT, O_TILE], BF16, kind="ExternalInput")
    idxr = nc.dram_tensor("idxr", [1, N_SPARSE], mybir.dt.float32r,
                          kind="ExternalInput")
    onesr = nc.dram_tensor("onesr", [1, P], mybir.dt.float32r,
                           kind="ExternalInput")
    out = nc.dram_tensor("out", [NCH, TPC, P, N_OUT], BF16,
                         kind="ExternalOutput")

    with tile.TileContext(nc) as tc:
        with (
            tc.tile_pool(name="const", bufs=1) as const_pool,
            tc.tile_pool(name="gpool", bufs=1) as g_pool,
            tc.tile_pool(name="xgpool", bufs=1) as xg_pool,
            tc.tile_pool(name="xin", bufs=2) as x_pool,
            tc.tile_pool(name="wpool", bufs=1) as wt_pool,
            tc.tile_pool(name="opool", bufs=2) as o_pool,
            tc.tile_pool(name="ps_g", bufs=3, space="PSUM") as psg,
            tc.tile_pool(name="ps_b", bufs=5, space="PSUM") as psb,
        ):
            warm_sb = const_pool.tile([P, O_TILE], BF16)
            nc.gpsimd.memset(warm_sb[:], 0.0)
            # x_gT resident: [k-part, kt, m] bf16
            xg_sb = xg_pool.tile([P, NKT, M], BF16)
            g_sb = g_pool.tile([P, nb_total, P], BF16)
            wt_sb = wt_pool.tile([P, NOS, NKT, O_TILE], BF16)
            x_tiles = [
                x_pool.tile([P, NNB, MC], BF16, tag="xin", name=f"x{c}")
                for c in range(NCH)
            ]

            F32R = mybir.dt.float32r
            idxr_sb = const_pool.tile([1, N_SPARSE], F32R)
            idxf_sb = const_pool.tile([P, N_SPARSE], F32)
            ones_sb = const_pool.tile([1, P], F32R)
            iota_sb = const_pool.tile([P, NNB], F32)
            # value at (p, b) = p + 128*b; f32 is exact here (max 4223)
            nc.gpsimd.iota(
                iota_sb[:], pattern=[[P, NNB]], base=0, channel_multiplier=1,
                allow_small_or_imprecise_dtypes=True,
            )

            # ---- DMA schedule (two HWDGE rings, ~200-225 B/ns each) ----
            # sync:   c0, wt3, wt5, wt7, xc2, (stores)
            # scalar: idxr, onesr, wt0, wt1, wt2, wt4, wt6, xc1, xc3
            nc.scalar.dma_start(idxr_sb[:], idxr[:])
            nc.scalar.dma_start(ones_sb[:], onesr[:])
            nc.sync.dma_start(x_tiles[0][:], x[0])
            for s, eng in [(0, nc.scalar), (1, nc.scalar), (2, nc.scalar),
                           (3, nc.sync), (4, nc.scalar), (5, nc.sync),
                           (6, nc.scalar), (7, nc.sync)]:
                eng.dma_start(wt_sb[:, s], wt[s])
            nc.scalar.dma_start(x_tiles[1][:], x[1])
            nc.sync.dma_start(x_tiles[2][:], x[2])
            nc.scalar.dma_start(x_tiles[3][:], x[3])

            # The PE's first work replicates sparse_idx across all 128
            # partitions (ones[1,128].T @ idx[1,512]) for the DVE one-hot
            # synthesis - its wait on the tiny idxr DMA lands in the cold
            # window anyway.  Then memset-tile warmups (no DMA dependency)
            # fill the rest of the x-load window so the HAM-gated clock is
            # at 2.4 GHz when the real stream begins.
            for h in range(2):
                bps = psb.tile([P, O_TILE], F32, tag="psb", name=f"bcast{h}")
                nc.tensor.matmul(
                    bps[:],
                    ones_sb[:],
                    idxr_sb[:, h * O_TILE:(h + 1) * O_TILE],
                    start=True,
                    stop=True,
                )
                nc.scalar.copy(
                    idxf_sb[:, h * O_TILE:(h + 1) * O_TILE], bps[:]
                )
            for w in range(N_WARM):
                wps = psb.tile([P, O_TILE], F32, tag="psb", name=f"warm{w}")
                nc.tensor.matmul(
                    wps[:], warm_sb[:, :P], warm_sb[:], start=True, stop=True
                )

            # synthesize the one-hot gather blocks on DVE, k-tile order so
            # the first gathers never wait: G[n, krel] = (idx[krel] == n+128b)
            for kt in range(NKT):
                for bi, b in blocks_per_kt[kt]:
                    nc.vector.tensor_tensor(
                        out=g_sb[:, bi, :],
                        in0=iota_sb[:, b:b + 1].to_broadcast([P, P])[:],
                        in1=idxf_sb[:, kt * P:(kt + 1) * P],
                        op=mybir.AluOpType.is_equal,
                    )

            for c in range(NCH):
                x_sb = x_tiles[c]
                m0 = c * MC
                last = c == NCH - 1
                # ---- gather n->k for this m-chunk ----
                for kt in range(NKT):
                    entries = blocks_per_kt[kt]
                    ps = psg.tile([P, MC], F32, tag="psg", name=f"psg{c}_{kt}")
                    for i, (bi, b) in enumerate(entries):
                        nc.tensor.matmul(
                            ps[:],
                            g_sb[:, bi, :],
                            x_sb[:, b, :],
                            start=(i == 0),
                            stop=(i == len(entries) - 1),
                        )
                    nc.scalar.copy(xg_sb[:, kt, m0:m0 + MC], ps[:])

                # ---- GEMM for this m-chunk ----
                slab = o_pool.tile([P, TPC, N_OUT], BF16, tag="ob",
                                   name=f"ob{c}")
                for s in range(NOS):
                    for t in range(TPC):
                        ps = psb.tile([P, O_TILE], F32, tag="psb",
                                      name=f"psb{c}_{s}_{t}")
                        mt0 = m0 + t * P
                        for kt in range(NKT):
                            nc.tensor.matmul(
                                ps[:],
                                xg_sb[:, kt, mt0:mt0 + P],
                                wt_sb[:, s, kt, :],
                                start=(kt == 0),
                                stop=(kt == NKT - 1),
                            )
                        dst = slab[:, t, s * O_TILE:(s + 1) * O_TILE]
                        # split the final chunk's evictions across DVE and
                        # ACT so the drain tail stays short
                        if last and s % 2 == 1:
                            nc.scalar.copy(dst, ps[:])
                        else:
                            nc.vector.tensor_copy(dst, ps[:])
                # store per (t, o-half), alternating rings; the final
                # chunk stores in o-quarters so the drain tail is short
                nq = 4 if last else 2
                for t in range(TPC):
                    for h in range(nq):
                        o0 = h * (N_OUT // nq)
                        eng = nc.sync if (t + h) % 2 == 0 else nc.scalar
                        eng.dma_start(
                            out[c, t, :, o0:o0 + N_OUT // nq],
                            slab[:, t, o0:o0 + N_OUT // nq],
                        )
    nc.compile()
    return nc


def _get_compiled(idx: np.ndarray):
    key = idx.tobytes()
    if key not in _cache:
        g_all, blocks_per_kt = _build_gather_blocks(idx)
        nc = _build_nc(blocks_per_kt, g_all.shape[0])
        _cache[key] = (nc, g_all)
    return _cache[key]


def _run(inputs, trace=False, trace_kwargs=None):
    import ml_dtypes
    from concourse.bass_utils import run_bass_kernel_spmd

    BF = ml_dtypes.bfloat16

    x = np.asarray(inputs["x"], dtype=np.float32)
    sv = np.asarray(inputs["sparse_values"], dtype=np.float32)
    idx = np.asarray(inputs["sparse_idx"]).astype(np.int64)

    nc, g_all = _get_compiled(idx)

    x2 = x.reshape(M_TOT, N_IN).astype(BF)
    # wt swizzled for contiguous per-partition DMA: [o-slice, k%128, k//128, o]
    wtv = np.ascontiguousarray(
        sv.T.reshape(NKT, P, NOS, O_TILE).transpose(2, 1, 0, 3).astype(BF)
    )
    # sparse_idx as a single f32 row; the PE replicates it on device
    idxr = np.ascontiguousarray(idx.astype(np.float32)[None, :])
    onesr = np.ones((1, P), dtype=np.float32)
    in_maps = []
    for c in range(N_CORES):
        xs = x2[c * M:(c + 1) * M]  # [1024, 4096] bf16
        # [chunk, part(n%128), n-block, m]: orig dims [c, m, b, p]
        xswz = np.ascontiguousarray(
            xs.reshape(NCH, MC, NNB, P).transpose(0, 3, 2, 1)
        )
        in_maps.append({"x": xswz, "wt": wtv, "idxr": idxr,
                        "onesr": onesr})
    res = run_bass_kernel_spmd(
        nc,
        in_maps,
        core_ids=list(range(N_CORES)),
        trace=trace,
        **(trace_kwargs or {}),
    )
    full = np.concatenate(
        [np.asarray(r["out"]).astype(np.float32).reshape(M, N_OUT)
         for r in res.results],
        axis=0,
    )
    return full.reshape(B, S, N_OUT), res


def kernel(**inputs) -> np.ndarray:
    out, _ = _run(inputs)
    return out


# revision 16
# speedup vs baseline: 1.0145x; 1.0145x over previous
"""Trainium2 Bass kernel for structured-sparse matmul.

Computes: out[b,s,o] = sum_k x[b,s,sparse_idx[k]] * sparse_values[o,k]
  x: [4, 2048, 4096] f32, sparse_values: [4096, 1024] f32,
  sparse_idx: [1024] int64 (sorted, unique) -> out [4, 2048, 4096] f32

Strategy (8 NeuronCores, data-parallel over rows m = b*s, bf16 compute):
  Host-side layout prep only (like the wt swizzle): x slice is transposed
  and swizzled to [chunk, part, n-block, m] bf16 so every DMA moves large
  contiguous per-partition lines.  sparse_idx expands into one-hot
  selection blocks G (compile-time metadata).
  Per core (M=1024 rows), per m-chunk of 256:
    gather n->k via PE matmuls with G (xg_T[k, m] bf16), then GEMM
    out[m, o] = xg_T.T @ W^T[k, o] accumulating 8 k-tiles in PSUM
    (bf16 = 1 col/cycle = the PE roofline here: fp8 DoubleRow measures
    2x but the 2e-2 accuracy gate forces a 3-term split = 1.5x bf16
    cost; GPSIMD ap_gather measures ~3 Gelem/s - 50x too slow), evict
    f32->bf16 (DVE, + ACT on the last chunk) and store bf16 per o-half
    (host upcasts).
  The G blocks are synthesized ON DEVICE (gpsimd iota + DVE is_equal
  against the replicated sparse_idx) so only 512KB of idx data rides the
  critical DMA prefix instead of 1.2MB of one-hot blocks, and wt0 can
  lead the scalar ring.  The PE clock (HAM-gated, 0.65->2.4 GHz over
  ~3.4us of activity) is kept warm by a long run of small matmuls on a
  gpsimd-memset tile - no DMA dependency - filling the whole x-load
  window so the real stream never re-ramps.
"""

import sys

if "/opt/trn_rl_repo" not in sys.path:
    sys.path.insert(0, "/opt/trn_rl_repo")

import numpy as np

B, S, N_IN = 4, 2048, 4096
N_OUT, N_SPARSE = 4096, 1024
N_CORES = 8
M_TOT = B * S            # 8192
M = M_TOT // N_CORES     # 1024 rows per core
P = 128
NKT = N_SPARSE // P      # 8 k-tiles
NNB = N_IN // P          # 32 n-blocks
MC = 256                 # m-chunk for gather/GEMM pipelining
NCH = M // MC            # 4 chunks
TPC = MC // P            # 2 m-tiles per chunk
O_TILE = 512
NOS = N_OUT // O_TILE    # 8 o-slices
N_WARM = 12

_cache: dict = {}


def _build_gather_blocks(idx: np.ndarray):
    """Expand sparse_idx into one-hot selection blocks.

    For k-tile kt and n-block b, G[n, krel] = 1 iff idx[kt*128+krel] == b*128+n.
    Returns (g_all [NB,128,128] f32, blocks_per_kt: list of lists of (bi, b)).
    """
    mats = []
    blocks_per_kt = []
    for kt in range(NKT):
        ks = idx[kt * P:(kt + 1) * P]
        bs = sorted(set(int(k) // P for k in ks))
        entries = []
        for b in bs:
            mat = np.zeros((P, P), dtype=np.float32)
            for krel, k in enumerate(ks):
                if int(k) // P == b:
                    mat[int(k) % P, krel] = 1.0
            entries.append((len(mats), b))
            mats.append(mat)
        blocks_per_kt.append(entries)
    return np.stack(mats), blocks_per_kt


def _build_nc(blocks_per_kt, nb_total):
    import concourse.mybir as mybir
    import concourse.tile as tile
    from concourse import bacc

    BF16 = mybir.dt.bfloat16
    F32 = mybir.dt.float32

    nc = bacc.Bacc("TRN2", target_bir_lowering=False, debug=False)
    # x swizzled on host: [chunk, part(n%128), n-block, m] bf16
    x = nc.dram_tensor("x", [NCH, P, NNB, MC], BF16, kind="ExternalInput")
    wt = nc.dram_tensor("wt", [NOS, P, NKT, O_TILE], BF16, kind="ExternalInput")
    idxr = nc.dram_tensor("idxr", [1, N_SPARSE], mybir.dt.float32r,
                          kind="ExternalInput")
    onesr = nc.dram_tensor("onesr", [1, P], mybir.dt.float32r,
                           kind="ExternalInput")
    out = nc.dram_tensor("out", [NCH, TPC, P, N_OUT], BF16,
                         kind="ExternalOutput")

    with tile.TileContext(nc) as tc:
        with (
            tc.tile_pool(name="const", bufs=1) as const_pool,
            tc.tile_pool(name="gpool", bufs=1) as g_pool,
            tc.tile_pool(name="xgpool", bufs=1) as xg_pool,
            tc.tile_pool(name="xin", bufs=2) as x_pool,
            tc.tile_pool(name="wpool", bufs=1) as wt_pool,
            tc.tile_pool(name="opool", bufs=2) as o_pool,
            tc.tile_pool(name="ps_g", bufs=3, space="PSUM") as psg,
            tc.tile_pool(name="ps_b", bufs=5, space="PSUM") as psb,
        ):
            warm_sb = const_pool.tile([P, O_TILE], BF16)
            nc.gpsimd.memset(warm_sb[:], 0.0)
            # x_gT resident: [k-part, kt, m] bf16
            xg_sb = xg_pool.tile([P, NKT, M], BF16)
            g_sb = g_pool.tile([P, nb_total, P], BF16)
            wt_sb = wt_pool.tile([P, NOS, NKT, O_TILE], BF16)
            x_tiles = [
                x_pool.tile([P, NNB, MC], BF16, tag="xin", name=f"x{c}")
                for c in range(NCH)
            ]

            F32R = mybir.dt.float32r
            idxr_sb = const_pool.tile([1, N_SPARSE], F32R)
            idxf_sb = const_pool.tile([P, N_SPARSE], F32)
            ones_sb = const_pool.tile([1, P], F32R)
            iota_sb = const_pool.tile([P, NNB], F32)
            # value at (p, b) = p + 128*b; f32 is exact here (max 4223)
            nc.gpsimd.iota(
                iota_sb[:], pattern=[[P, NNB]], base=0, channel_multiplier=1,
                allow_small_or_imprecise_dtypes=True,
            )

            # ---- DMA schedule (two HWDGE rings, ~200-225 B/ns each) ----
            # sync:   c0, wt3, wt5, wt7, xc2, (stores)
            # scalar: idxr, onesr, wt0, wt1, wt2, wt4, wt6, xc1, xc3
            nc.scalar.dma_start(idxr_sb[:], idxr[:])
            nc.scalar.dma_start(ones_sb[:], onesr[:])
            nc.sync.dma_start(x_tiles[0][:], x[0])
            for s, eng in [(0, nc.scalar), (1, nc.scalar), (2, nc.scalar),
                           (3, nc.sync), (4, nc.scalar), (5, nc.sync),
                           (6, nc.scalar), (7, nc.sync)]:
                eng.dma_start(wt_sb[:, s], wt[s])
            nc.scalar.dma_start(x_tiles[1][:], x[1])
            nc.sync.dma_start(x_tiles[2][:], x[2])
            nc.scalar.dma_start(x_tiles[3][:], x[3])

            # The PE's first work replicates sparse_idx across all 128
            # partitions (ones[1,128].T @ idx[1,512]) for the DVE one-hot
            # synthesis - its wait on the tiny idxr DMA lands in the cold
            # window anyway.  Then memset-tile warmups (no DMA dependency)
            # fill the rest of the x-load window so the HAM-gated clock is
            # at 2.4 GHz when the real stream begins.
            for h in range(2):
                bps = psb.tile([P, O_TILE], F32, tag="psb", name=f"bcast{h}")
                nc.tensor.matmul(
                    bps[:],
                    ones_sb[:],
                    idxr_sb[:, h * O_TILE:(h + 1) * O_TILE],
                    start=True,
                    stop=True,
                )
                nc.scalar.copy(
                    idxf_sb[:, h * O_TILE:(h + 1) * O_TILE], bps[:]
                )
            for w in range(N_WARM):
                wps = psb.tile([P, O_TILE], F32, tag="psb", name=f"warm{w}")
                nc.tensor.matmul(
                    wps[:], warm_sb[:, :P], warm_sb[:], start=True, stop=True
                )

            # synthesize the one-hot gather blocks on DVE, k-tile order so
            # the first gathers never wait: G[n, krel] = (idx[krel] == n+128b)
            for kt in range(NKT):
                for bi, b in blocks_per_kt[kt]:
                    nc.vector.tensor_tensor(
                        out=g_sb[:, bi, :],
                        in0=iota_sb[:, b:b + 1].to_broadcast([P, P])[:],
                        in1=idxf_sb[:, kt * P:(kt + 1) * P],
                        op=mybir.AluOpType.is_equal,
                    )

            for c in range(NCH):
                x_sb = x_tiles[c]
                m0 = c * MC
                last = c == NCH - 1
                # ---- gather n->k for this m-chunk ----
                for kt in range(NKT):
                    entries = blocks_per_kt[kt]
                    ps = psg.tile([P, MC], F32, tag="psg", name=f"psg{c}_{kt}")
                    for i, (bi, b) in enumerate(entries):
                        nc.tensor.matmul(
                            ps[:],
                            g_sb[:, bi, :],
                            x_sb[:, b, :],
                            start=(i == 0),
                            stop=(i == len(entries) - 1),
                        )
                    nc.scalar.copy(xg_sb[:, kt, m0:m0 + MC], ps[:])

                # ---- GEMM for this m-chunk ----
                slab = o_pool.tile([P, TPC, N_OUT], BF16, tag="ob",
                                   name=f"ob{c}")
                for s in range(NOS):
                    for t in range(TPC):
                        ps = psb.tile([P, O_TILE], F32, tag="psb",
                                      name=f"psb{c}_{s}_{t}")
                        mt0 = m0 + t * P
                        for kt in range(NKT):
                            nc.tensor.matmul(
                                ps[:],
                                xg_sb[:, kt, mt0:mt0 + P],
                                wt_sb[:, s, kt, :],
                                start=(kt == 0),
                                stop=(kt == NKT - 1),
                            )
                        dst = slab[:, t, s * O_TILE:(s + 1) * O_TILE]
                        # split the final chunk's evictions across DVE and
                        # ACT so the drain tail stays short
                        if last and s % 2 == 1:
                            nc.scalar.copy(dst, ps[:])
                        else:
                            nc.vector.tensor_copy(dst, ps[:])
                # store per (t, o-half), alternating rings
                for t in range(TPC):
                    for h in range(2):
                        o0 = h * (N_OUT // 2)
                        eng = nc.sync if (t + h) % 2 == 0 else nc.scalar
                        eng.dma_start(
                            out[c, t, :, o0:o0 + N_OUT // 2],
                            slab[:, t, o0:o0 + N_OUT // 2],
                        )
    nc.compile()
    return nc


def _get_compiled(idx: np.ndarray):
    key = idx.tobytes()
    if key not in _cache:
        g_all, blocks_per_kt = _build_gather_blocks(idx)
        nc = _build_nc(blocks_per_kt, g_all.shape[0])
        _cache[key] = (nc, g_all)
    return _cache[key]


def _run(inputs, trace=False, trace_kwargs=None):
    import ml_dtypes
    from concourse.bass_utils import run_bass_kernel_spmd

    BF = ml_dtypes.bfloat16

    x = np.asarray(inputs["x"], dtype=np.float32)
    sv = np.asarray(inputs["sparse_values"], dtype=np.float32)
    idx = np.asarray(inputs["sparse_idx"]).astype(np.int64)

    nc, g_all = _get_compiled(idx)

    x2 = x.reshape(M_TOT, N_IN).astype(BF)
    # wt swizzled for contiguous per-partition DMA: [o-slice, k%128, k//128, o]
    wtv = np.ascontiguousarray(
        sv.T.reshape(NKT, P, NOS, O_TILE).transpose(2, 1, 0, 3).astype(BF)
    )
    # sparse_idx as a single f32 row; the PE replicates it on device
    idxr = np.ascontiguousarray(idx.astype(np.float32)[None, :])
    onesr = np.ones((1, P), dtype=np.float32)
    in_maps = []
    for c in range(N_CORES):
        xs = x2[c * M:(c + 1) * M]  # [1024, 4096] bf16
        # [chunk, part(n%128), n-block, m]: orig dims [c, m, b, p]
        xswz = np.ascontiguousarray(
            xs.reshape(NCH, MC, NNB, P).transpose(0, 3, 2, 1)
        )
        in_maps.append({"x": xswz, "wt": wtv, "idxr": idxr,
                        "onesr": onesr})
    res = run_bass_kernel_spmd(
        nc,
        in_maps,
        core_ids=list(range(N_CORES)),
        trace=trace,
        **(trace_kwargs or {}),
    )
    full = np.concatenate(
        [np.asarray(r["out"]).astype(np.float32).reshape(M, N_OUT)
         for r in res.results],
        axis=0,
    )
    return full.reshape(B, S, N_OUT), res


def kernel(**inputs) -> np.ndarray:
    out, _ = _run(inputs)
    return out
